# revision 28
# baseline (speedup 1.0000x reference)
"""Distance transform via per-radius box-sums, width-adaptive schedule.

D(p) = Chebyshev distance to nearest source. boxsum_t = clamped (2t+1)^2
window sum (monotone in t); b_t = [boxsum_t>0]; Sum_{t<=T} b_t = T+1-D.
Per t the box sum is separable: row-window from host prefix-sum differences,
column-window via banded matmul W_t. The row-window subtract is folded into
the PE: ps_t = W_t @ P[right] + W_t @ (-P)[left] (two accumulating fp8
matmuls, no vector subtract). Signs: ACT triad/pair/quad ops or DVE fused
(ps>0)+F chains. Images are sorted by per-image iteration count n_i and
dealt to (core, slab) so slab widths shrink 384->256->128 as slabs retire.

Reconstruction (exact): s* = sum_{8-neighb q, clamped} K(q-p)[D(q)<D(p)];
out = (D-1) - h*log(s*) = -F - h*ln(w2*(r*C4+C8)) with F = Sum b - T_slab
accumulated as exact integers in bf16. r*C4+C8 is accumulated on the PE
(identity / r*identity matmuls over the mask tiles) straight into PSUM.
"""

import math

import numpy as np

H_PARAM = 0.35
W1 = math.exp(-1.0 / H_PARAM)
W2 = math.exp(-math.sqrt(2.0) / H_PARAM)
_PROGRAM_CACHE = {}


def _needed_iters_per_image(flat):
    B = flat > 0
    n = np.zeros(B.shape[0], dtype=np.int64)
    live = ~B.all(axis=(1, 2))
    it = 0
    while live.any() and it < 128:
        P = np.pad(B, ((0, 0), (1, 1), (1, 1)), mode="edge")
        D = np.zeros_like(B)
        for dy in range(3):
            for dx in range(3):
                D |= P[:, dy : dy + 128, dx : dx + 128]
        B = D
        it += 1
        n[live] += 1
        live = ~B.all(axis=(1, 2))
    return n


def _schedule(t3e, t2e, t1e):
    """Per-t width (slabs) and sign-group structure.

    Groups: ("act", [t...]) one ACT sign op over the group's PSUM tile;
    ("dve", [t]) fused (ps>0)+chain on DVE. The DVE chain must END before
    the last few ACT groups so slab finals are ready early.
    """
    width = {}
    for t in range(1, t1e + 1):
        width[t] = 3 if t <= t3e else (2 if t <= t2e else 1)
    groups = []
    w3 = list(range(1, t3e + 1))
    w2ts = list(range(t3e + 1, t2e + 1))
    w1ts = list(range(t2e + 1, t1e + 1))
    # w3: alternate ACT triads with DVE runs (PSUM: one triad in flight)
    i = 0
    toggle = True
    while i < len(w3):
        if toggle and len(w3) - i >= 3:
            groups.append(("act", w3[i : i + 3]))
            i += 3
        else:
            k = min(4, len(w3) - i)
            for t in w3[i : i + k]:
                groups.append(("dve", [t]))
            i += k
        toggle = not toggle
    # w2: first t on the DVE chain (ends the wide chain early), rest ACT
    if w2ts:
        groups.append(("dve", [w2ts[0]]))
        rest = w2ts[1:]
        i = 0
        while i < len(rest):
            if len(rest) - i >= 2:
                groups.append(("act", rest[i : i + 2]))
                i += 2
            else:
                groups.append(("act", [rest[i]]))
                i += 1
    # w1: ACT quads; their sub-signs are PE-accumulated (no DVE chain)
    i = 0
    while i < len(w1ts):
        k = min(4, len(w1ts) - i)
        if k >= 2:
            groups.append(("act", w1ts[i : i + k]))
        else:
            groups.append(("dve", [w1ts[i]]))
        i += k
    return width, groups


def _make_wmats(t1e):
    i = np.arange(128)
    d = np.abs(i[:, None] - i[None, :])
    W = np.stack([(d <= t) for t in range(1, t1e + 1)])
    return np.ascontiguousarray(W.transpose(1, 0, 2).reshape(128, t1e * 128))


def _make_shifts():
    m = np.arange(128)
    ShU_T = np.zeros((128, 128), dtype=np.float32)
    ShU_T[np.maximum(m - 1, 0), m] = 1
    ShD_T = np.zeros((128, 128), dtype=np.float32)
    ShD_T[np.minimum(m + 1, 127), m] = 1
    I = np.eye(128, dtype=np.float32)
    return np.stack([ShU_T, ShD_T, I, (W1 / W2) * I])


def _build(t3e, t2e, t1e):
    import concourse.bacc as bacc
    import concourse.tile as tile
    from concourse import mybir
    from concourse.alu_op_type import AluOpType as alu

    f32 = mybir.dt.float32
    bf16 = mybir.dt.bfloat16
    f8 = mybir.dt.float8e4

    PL = t1e + 1
    WIM = PL + 128 + t1e
    FWP = 3 * WIM
    NW = t1e * 128
    width, groups = _schedule(t3e, t2e, t1e)

    nc = bacc.Bacc(
        "TRN2",
        target_bir_lowering=False,
        debug=False,
        enable_asserts=False,
        num_devices=8,
    )
    pd = nc.dram_tensor("p", [128, FWP], f8, kind="ExternalInput")
    npd = nc.dram_tensor("np", [128, FWP], f8, kind="ExternalInput")
    d0d = nc.dram_tensor("d0", [128, 384], bf16, kind="ExternalInput")
    wd = nc.dram_tensor("w", [128, NW], f8, kind="ExternalInput")
    shd = nc.dram_tensor("sh", [4, 128, 128], bf16, kind="ExternalInput")
    outd = nc.dram_tensor("out", [3, 128, 128], f32, kind="ExternalOutput")

    with tile.TileContext(nc) as tc:
        with (
            tc.tile_pool(name="state", bufs=1) as st,
            tc.tile_pool(name="work", bufs=1) as wk,
            tc.tile_pool(name="psq", bufs=1, space="PSUM") as pq,
            tc.tile_pool(name="psq4", bufs=3, space="PSUM") as pq4,
            tc.tile_pool(name="pss", bufs=2, space="PSUM") as ps_pool,
        ):
            P = st.tile([128, FWP], f8, name="P")
            NP = st.tile([128, FWP], f8, name="NP")
            D0 = st.tile([128, 384], bf16, name="D0")
            Ws = st.tile([128, NW], f8, name="Ws")
            Sh = st.tile([128, 4 * 128], bf16, name="Sh")

            # --- DMA prologue spread over three queues: sync (W + P),
            # scalar (NP, before its table load), gpsimd (rest).
            c1 = min(2, t1e)
            nc.sync.dma_start(Ws[:, : c1 * 128], wd.ap()[:, : c1 * 128])
            nc.scalar.dma_start(NP[:], npd.ap())
            nc.sync.dma_start(P[:], pd.ap())
            c2 = min(13, t1e)
            cm = min(7, t1e)
            if cm > c1:
                nc.sync.dma_start(
                    Ws[:, c1 * 128 : cm * 128],
                    wd.ap()[:, c1 * 128 : cm * 128],
                )
            if c2 > cm:
                nc.scalar.dma_start(
                    Ws[:, cm * 128 : c2 * 128],
                    wd.ap()[:, cm * 128 : c2 * 128],
                )
            nc.gpsimd.dma_start(D0[:], d0d.ap())
            if t1e > c2:
                nc.gpsimd.dma_start(
                    Ws[:, c2 * 128 :], wd.ap()[:, c2 * 128 :]
                )
            nc.gpsimd.dma_start(
                Sh[:].rearrange("k (t m) -> k t m", t=4),
                shd.ap().rearrange("t k m -> k t m"),
            )

            # --- ACT table warm (Ln first: its set covers Sign/Copy) ---
            warm = wk.tile([128, 1], f32, tag="warm")
            nc.vector.memset(warm[:], 1.0)
            warm2 = wk.tile([128, 1], f32, tag="warm2")
            nc.scalar.activation(
                warm2[:], warm[:], mybir.ActivationFunctionType.Ln
            )
            lnbias = st.tile([128, 1], f32, name="lnbias")
            nc.gpsimd.memset(lnbias[:], 1e-30)

            # --- PE HAM warm-up: a few junk matmuls that finish before the
            # first real matmul's inputs land (PE queue is FIFO) ---
            junk = st.tile([128, 512], bf16, name="junk")
            nc.vector.memset(junk[:], 0.0)
            for _ in range(5):
                jps = ps_pool.tile([128, 512], f32, tag="s")
                nc.tensor.matmul(
                    jps[:], junk[:, 0:128], junk[:], start=True, stop=True
                )



            Pv = P[:].rearrange("p (c w) -> p c w", c=3)
            NPv = NP[:].rearrange("p (c w) -> p c w", c=3)

            def mm_pair(ps, off, t, w):
                """ps[:, off:off+128w] += W_t @ (P[right] - P[left])."""
                wsl = Ws[:, (t - 1) * 128 : t * 128]
                r0 = PL + t
                l0 = PL - t - 1
                nc.tensor.matmul(
                    ps[:, off : off + 128 * w],
                    wsl,
                    Pv[:, 0:w, r0 : r0 + 128],
                    start=True,
                    stop=False,
                )
                nc.tensor.matmul(
                    ps[:, off : off + 128 * w],
                    wsl,
                    NPv[:, 0:w, l0 : l0 + 128],
                    start=False,
                    stop=True,
                )

            # --- phase 1 ---
            sgn = mybir.ActivationFunctionType.Sign
            Fcur = D0
            F_by_w = {}
            act_parts = []  # (folded tile, width_units)

            for kind, ts in groups:
                w = width[ts[0]]
                u = 128 * w
                if kind == "act":
                    g = len(ts)
                    stride = 512 if u > 256 else u
                    pool = pq if g * stride > 1024 else pq4
                    ps = pool.tile(
                        [128, g * stride], f32, tag=f"q{g * stride}"
                    )
                    for j, t in enumerate(ts):
                        mm_pair(ps, j * stride, t, w)
                    b = wk.tile([128, g * u], bf16, tag=f"b{ts[0]}")
                    if stride == u:
                        nc.scalar.activation(b[:], ps[:], sgn)
                    else:
                        nc.scalar.activation(
                            b[:].rearrange("p (g w) -> p g w", g=g),
                            ps[:].rearrange("p (g w) -> p g w", g=g)[
                                :, :, 0:u
                            ],
                            sgn,
                        )
                    if u == 128:
                        # w1 groups: defer, sub-signs get PE-accumulated
                        act_parts.append((b, -g))
                        continue
                    # fold to one u-wide partial (DVE: GpS is ~2.3x slower
                    # and its serial chain gated the tail)
                    feng = nc.vector
                    if g == 1:
                        cur = b
                    elif g == 2:
                        cur = wk.tile([128, u], bf16, tag=f"f{ts[0]}_0")
                        feng.tensor_tensor(
                            cur[:], b[:, 0:u], b[:, u : 2 * u], op=alu.add
                        )
                    elif g == 3:
                        f0 = wk.tile([128, u], bf16, tag=f"f{ts[0]}_0")
                        feng.tensor_tensor(
                            f0[:], b[:, u : 2 * u], b[:, 2 * u : 3 * u],
                            op=alu.add,
                        )
                        cur = wk.tile([128, u], bf16, tag=f"f{ts[0]}_1")
                        feng.tensor_tensor(
                            cur[:], b[:, 0:u], f0[:], op=alu.add
                        )
                    else:  # g == 4
                        f0 = wk.tile([128, 2 * u], bf16, tag=f"f{ts[0]}_0")
                        feng.tensor_tensor(
                            f0[:], b[:, 0 : 2 * u], b[:, 2 * u : 4 * u],
                            op=alu.add,
                        )
                        cur = wk.tile([128, u], bf16, tag=f"f{ts[0]}_1")
                        feng.tensor_tensor(
                            cur[:], f0[:, 0:u], f0[:, u : 2 * u], op=alu.add
                        )
                    act_parts.append((cur, w))
                else:
                    t = ts[0]
                    ps = ps_pool.tile([128, 512], f32, tag="s")
                    mm_pair(ps, 0, t, w)
                    nxt = wk.tile([128, u], bf16, tag=f"Fc{t}")
                    nc.vector.scalar_tensor_tensor(
                        nxt[:],
                        ps[:, 0:u],
                        0.0,
                        Fcur[:, 0:u],
                        op0=alu.is_gt,
                        op1=alu.add,
                    )
                    Fcur = nxt
                    F_by_w[w] = nxt

            F384 = F_by_w.get(3, D0)
            F256 = F_by_w.get(2, F384)
            F128 = F_by_w.get(1, F256)
            parts3 = [p for p, pw in act_parts if pw == 3]
            parts2 = [p for p, pw in act_parts if pw == 2]
            parts1 = [p for p, pw in act_parts if pw == 1]
            w1subs = [
                (p, -pw) for p, pw in act_parts if pw < 0
            ]  # (tile of g compact 128-wide sub-signs, g)

            # --- merge: presum ACT parts per slab range, then one add with
            # the chain state writes straight into Fg slices ---
            def tree(tiles, lo, hi, ei, tag):
                eng = [nc.vector, nc.vector]
                cur = [t[:, lo:hi] for t in tiles]
                k = 0
                while len(cur) > 1:
                    nxt = []
                    for i in range(0, len(cur) - 1, 2):
                        o = wk.tile(
                            [128, hi - lo], bf16, tag=f"{tag}{k}_{i}"
                        )
                        eng[(ei + i // 2) % 2].tensor_tensor(
                            o[:], cur[i], cur[i + 1], op=alu.add
                        )
                        nxt.append(o[:])
                    if len(cur) % 2:
                        nxt.append(cur[-1])
                    cur = nxt
                    k += 1
                return cur[0]

            Fg = st.tile([128, 384], bf16, name="Fg")
            # slab2 (cols 256:384): chain F384 + 384-wide parts
            E2 = tree(parts3, 256, 384, 0, "e2") if parts3 else None
            if E2 is not None:
                nc.vector.tensor_tensor(
                    Fg[:, 256:384], F384[:, 256:384], E2, op=alu.add
                )
            else:
                nc.vector.tensor_copy(Fg[:, 256:384], F384[:, 256:384])
            # slabs 0+1 share the ACT-part sum (parts1 is empty: w1 is on
            # the chain); E01 over cols 0:256 is ready by mid-run
            E01 = (
                tree(parts3 + parts2, 0, 256, 1, "e01")
                if parts3 + parts2
                else None
            )
            if E01 is not None:
                nc.vector.tensor_tensor(
                    Fg[:, 128:256], F256[:, 128:256], E01[:, 128:256],
                    op=alu.add,
                )
            else:
                nc.vector.tensor_copy(Fg[:, 128:256], F256[:, 128:256])
            if w1subs:
                # slab0: PE-accumulate w1 sub-signs + E01 + chain into PSUM
                I0 = Sh[:, 256:384]
                psF = ps_pool.tile([128, 512], f32, tag="s")
                first = True
                for b, g in w1subs:
                    for j in range(g):
                        nc.tensor.matmul(
                            psF[:, 0:128],
                            I0,
                            b[:, j * 128 : (j + 1) * 128],
                            start=first,
                            stop=False,
                        )
                        first = False
                if E01 is not None:
                    nc.tensor.matmul(
                        psF[:, 0:128], I0, E01[:, 0:128],
                        start=first, stop=False,
                    )
                    first = False
                nc.tensor.matmul(
                    psF[:, 0:128], I0, F128[:, 0:128],
                    start=first, stop=True,
                )
                nc.vector.tensor_scalar_add(
                    Fg[:, 0:128], psF[:, 0:128], 0.0
                )
            elif E01 is not None:
                nc.vector.tensor_tensor(
                    Fg[:, 0:128], F128[:, 0:128], E01[:, 0:128], op=alu.add
                )
            else:
                nc.vector.tensor_copy(Fg[:, 0:128], F128[:, 0:128])

            # --- phase 2 (single group, width 384) ---
            ln_fn = mybir.ActivationFunctionType.Ln
            cp_fn = mybir.ActivationFunctionType.Copy
            FW2 = 3 * 130
            Fp = st.tile([128, FW2], bf16, name="Fp")
            Fpv = Fp[:].rearrange("p (c w) -> p c w", c=3)
            Fgv = Fg[:].rearrange("p (c w) -> p c w", c=3)
            nc.vector.tensor_copy(Fpv[:, :, 1:129], Fgv)
            # edge columns replicate Fg cols 0/127 (reads Fg, not Fp, so it
            # runs in parallel with the pad copy)
            nc.scalar.activation(
                Fpv[:, :, 0:130:129], Fgv[:, :, 0:128:127], cp_fn
            )
            psU = ps_pool.tile([128, 512], f32, tag="s")
            nc.tensor.matmul(
                psU[:, 0:FW2], Sh[:, 0:128], Fp[:], start=True, stop=True
            )
            psD = ps_pool.tile([128, 512], f32, tag="s")
            nc.tensor.matmul(
                psD[:, 0:FW2], Sh[:, 128:256], Fp[:], start=True, stop=True
            )
            DU = st.tile([128, FW2], bf16, name="DU")
            nc.scalar.activation(DU[:], psU[:, 0:FW2], cp_fn)
            DD = st.tile([128, FW2], bf16, name="DD")
            nc.vector.tensor_scalar_add(DD[:], psD[:, 0:FW2], 0.0)
            DUv = DU[:].rearrange("p (c w) -> p c w", c=3)
            DDv = DD[:].rearrange("p (c w) -> p c w", c=3)

            def cmp(tp, name):
                o = wk.tile([128, 384], bf16, tag=name)
                nc.vector.tensor_tensor(
                    o[:].rearrange("p (c w) -> p c w", c=3),
                    tp,
                    Fgv,
                    op=alu.is_gt,
                )
                return o

            mL = cmp(Fpv[:, :, 0:128], "mL")
            mR = cmp(Fpv[:, :, 2:130], "mR")
            mU = cmp(DUv[:, :, 1:129], "mU")
            mD = cmp(DDv[:, :, 1:129], "mD")
            mUL = cmp(DUv[:, :, 0:128], "mUL")
            mUR = cmp(DUv[:, :, 2:130], "mUR")
            mDL = cmp(DDv[:, :, 0:128], "mDL")
            mDR = cmp(DDv[:, :, 2:130], "mDR")
            # sst = r*(mL+mR+mU+mD) + (mUL+mUR+mDL+mDR), accumulated on PE
            psS = ps_pool.tile([128, 512], f32, tag="s")
            rI = Sh[:, 384:512]
            I = Sh[:, 256:384]
            for j, m in enumerate([mL, mR, mU, mD]):
                nc.tensor.matmul(
                    psS[:, 0:384], rI, m[:], start=(j == 0), stop=False
                )
            for j, m in enumerate([mUL, mUR, mDL, mDR]):
                nc.tensor.matmul(
                    psS[:, 0:384], I, m[:], start=False, stop=(j == 3)
                )
            lnS = wk.tile([128, 384], f32, tag="lnS")
            nc.scalar.activation(
                lnS[:], psS[:, 0:384], ln_fn, bias=lnbias[:], scale=float(W2)
            )
            ut = wk.tile([128, 384], f32, tag="ut")
            nc.vector.scalar_tensor_tensor(
                ut[:], lnS[:], -H_PARAM, Fg[:],
                op0=alu.mult, op1=alu.subtract,
            )
            ov = wk.tile([128, 384], f32, tag="ov")
            nc.vector.scalar_tensor_tensor(
                ov[:], psS[:, 0:384], 0.0, ut[:],
                op0=alu.is_gt, op1=alu.mult,
            )
            nc.sync.dma_start(
                outd.ap().rearrange("c h w -> h c w"),
                ov[:].rearrange("p (c w) -> p c w", c=3),
            )

    nc.compile()
    return nc


def _get_program(key):
    if key not in _PROGRAM_CACHE:
        _PROGRAM_CACHE[key] = _build(*key)
    return _PROGRAM_CACHE[key]


def _prep_core(imgs, t3e, t2e, t1e):
    """imgs: [3,128,128] binary f32, slab-ordered. Returns input dict."""
    import ml_dtypes

    PL = t1e + 1
    WIM = PL + 128 + t1e
    x = (imgs > 0).astype(np.float64)
    Pr = np.cumsum(x, axis=-1)
    Ppad = np.zeros((3, 128, WIM), dtype=np.float64)
    Ppad[:, :, PL : PL + 128] = Pr
    Ppad[:, :, PL + 128 :] = Pr[:, :, 127:128]
    P8 = np.ascontiguousarray(
        Ppad.transpose(1, 0, 2).reshape(128, 3 * WIM)
    ).astype(np.float32)
    Ts = np.array([t1e, t2e, t3e], dtype=np.float64)
    D0 = x - Ts[:, None, None]
    D0 = np.ascontiguousarray(
        D0.transpose(1, 0, 2).reshape(128, 384)
    ).astype(ml_dtypes.bfloat16)
    return {
        "p": P8.astype(ml_dtypes.float8_e4m3fn),
        "np": (-P8).astype(ml_dtypes.float8_e4m3fn),
        "d0": D0,
        "w": _make_wmats(t1e).astype(ml_dtypes.float8_e4m3fn),
        "sh": _make_shifts().astype(ml_dtypes.bfloat16),
    }


def kernel(image):
    from concourse.bass_utils import run_bass_kernel_spmd

    image = np.ascontiguousarray(np.asarray(image), dtype=np.float32)
    assert image.shape == (8, 3, 128, 128)
    flat = image.reshape(24, 128, 128)
    ns = _needed_iters_per_image(flat)
    if ns.max() == 0:
        return np.zeros_like(image)
    order = np.argsort(-ns, kind="stable")
    t3e = int(ns[order[16]]) - 1
    t2e = int(ns[order[8]]) - 1
    t1e = int(ns[order[0]]) - 1
    t3e = max(t3e, 0)
    t2e = max(t2e, t3e)
    t1e = max(t1e, max(t2e, 1))
    nc = _get_program((t3e, t2e, t1e))
    in_maps = []
    for c in range(8):
        idx = [order[c], order[8 + c], order[16 + c]]
        in_maps.append(_prep_core(flat[idx], t3e, t2e, t1e))
    res = run_bass_kernel_spmd(nc, in_maps, core_ids=list(range(8)))
    out = np.zeros((24, 128, 128), dtype=np.float32)
    for c in range(8):
        o = res.results[c]["out"].astype(np.float32)
        for s in range(3):
            out[order[8 * s + c]] = o[s]
    return out.reshape(8, 3, 128, 128)


# revision 29
# speedup vs baseline: 1.0749x; 1.0749x over previous
"""Distance transform via per-radius box-sums, width-adaptive schedule.

D(p) = Chebyshev distance to nearest source. boxsum_t = clamped (2t+1)^2
window sum (monotone in t); b_t = [boxsum_t>0]; Sum_{t<=T} b_t = T+1-D.
Per t the box sum is separable: row-window from host prefix-sum differences,
column-window via banded matmul W_t. The row-window subtract is folded into
the PE: ps_t = W_t @ P[right] + W_t @ (-P)[left] (two accumulating fp8
matmuls, no vector subtract). Signs: ACT triad/pair/quad ops or DVE fused
(ps>0)+F chains. Images are sorted by per-image iteration count n_i and
dealt to (core, slab) so slab widths shrink 384->256->128 as slabs retire.

Reconstruction (exact): s* = sum_{8-neighb q, clamped} K(q-p)[D(q)<D(p)];
out = (D-1) - h*log(s*) = -F - h*ln(w2*(r*C4+C8)) with F = Sum b - T_slab
accumulated as exact integers in bf16. r*C4+C8 is accumulated on the PE
(identity / r*identity matmuls over the mask tiles) straight into PSUM.
"""

import math

import numpy as np

H_PARAM = 0.35
W1 = math.exp(-1.0 / H_PARAM)
W2 = math.exp(-math.sqrt(2.0) / H_PARAM)
_PROGRAM_CACHE = {}


def _needed_iters_per_image(flat):
    B = flat > 0
    n = np.zeros(B.shape[0], dtype=np.int64)
    live = ~B.all(axis=(1, 2))
    it = 0
    while live.any() and it < 128:
        P = np.pad(B, ((0, 0), (1, 1), (1, 1)), mode="edge")
        D = np.zeros_like(B)
        for dy in range(3):
            for dx in range(3):
                D |= P[:, dy : dy + 128, dx : dx + 128]
        B = D
        it += 1
        n[live] += 1
        live = ~B.all(axis=(1, 2))
    return n


def _schedule(t3e, t2e, t1e):
    """Per-t width (slabs) and sign-group structure.

    Groups: ("act", [t...]) one ACT sign op over the group's PSUM tile;
    ("dve", [t]) fused (ps>0)+chain on DVE. The DVE chain must END before
    the last few ACT groups so slab finals are ready early.
    """
    width = {}
    for t in range(1, t1e + 1):
        width[t] = 3 if t <= t3e else (2 if t <= t2e else 1)
    groups = []
    w3 = list(range(1, t3e + 1))
    w2ts = list(range(t3e + 1, t2e + 1))
    w1ts = list(range(t2e + 1, t1e + 1))
    # w3: alternate ACT triads with DVE runs (PSUM: one triad in flight)
    i = 0
    toggle = True
    while i < len(w3):
        if toggle and len(w3) - i >= 3:
            groups.append(("act", w3[i : i + 3]))
            i += 3
        else:
            k = min(4, len(w3) - i)
            for t in w3[i : i + k]:
                groups.append(("dve", [t]))
            i += k
        toggle = not toggle
    # w2: first t on the DVE chain (ends the wide chain early), rest ACT
    if w2ts:
        groups.append(("dve", [w2ts[0]]))
        rest = w2ts[1:]
        i = 0
        while i < len(rest):
            if len(rest) - i >= 2:
                groups.append(("act", rest[i : i + 2]))
                i += 2
            else:
                groups.append(("act", [rest[i]]))
                i += 1
    # w1: ACT quads; their sub-signs are PE-accumulated (no DVE chain)
    i = 0
    while i < len(w1ts):
        k = min(4, len(w1ts) - i)
        if k >= 2:
            groups.append(("act", w1ts[i : i + k]))
        else:
            groups.append(("dve", [w1ts[i]]))
        i += k
    return width, groups


def _make_wmats(t1e):
    i = np.arange(128)
    d = np.abs(i[:, None] - i[None, :])
    W = np.stack([(d <= t) for t in range(1, t1e + 1)])
    return np.ascontiguousarray(W.transpose(1, 0, 2).reshape(128, t1e * 128))


def _make_shifts():
    m = np.arange(128)
    ShU_T = np.zeros((128, 128), dtype=np.float32)
    ShU_T[np.maximum(m - 1, 0), m] = 1
    ShD_T = np.zeros((128, 128), dtype=np.float32)
    ShD_T[np.minimum(m + 1, 127), m] = 1
    I = np.eye(128, dtype=np.float32)
    return np.stack([ShU_T, ShD_T, I, (W1 / W2) * I])


def _build(t3e, t2e, t1e):
    import concourse.bacc as bacc
    import concourse.tile as tile
    from concourse import mybir
    from concourse.alu_op_type import AluOpType as alu

    f32 = mybir.dt.float32
    bf16 = mybir.dt.bfloat16
    f8 = mybir.dt.float8e4

    PL = t1e + 1
    WIM = PL + 128 + t1e
    FWP = 3 * WIM
    NW = t1e * 128
    width, groups = _schedule(t3e, t2e, t1e)

    nc = bacc.Bacc(
        "TRN2",
        target_bir_lowering=False,
        debug=False,
        enable_asserts=False,
        num_devices=8,
    )
    pd = nc.dram_tensor("p", [128, FWP], f8, kind="ExternalInput")
    npd = nc.dram_tensor("np", [128, FWP], f8, kind="ExternalInput")
    d0d = nc.dram_tensor("d0", [128, 384], bf16, kind="ExternalInput")
    wd = nc.dram_tensor("w", [128, NW], f8, kind="ExternalInput")
    shd = nc.dram_tensor("sh", [4, 128, 128], bf16, kind="ExternalInput")
    outd = nc.dram_tensor("out", [3, 128, 128], f32, kind="ExternalOutput")

    with tile.TileContext(nc) as tc:
        with (
            tc.tile_pool(name="state", bufs=1) as st,
            tc.tile_pool(name="work", bufs=1) as wk,
            tc.tile_pool(name="psq", bufs=1, space="PSUM") as pq,
            tc.tile_pool(name="psq4", bufs=3, space="PSUM") as pq4,
            tc.tile_pool(name="pss", bufs=2, space="PSUM") as ps_pool,
        ):
            P = st.tile([128, FWP], f8, name="P")
            NP = st.tile([128, FWP], f8, name="NP")
            D0 = st.tile([128, 384], bf16, name="D0")
            Ws = st.tile([128, NW], f8, name="Ws")
            Sh = st.tile([128, 4 * 128], bf16, name="Sh")

            # --- DMA prologue spread over three queues: sync (W + P),
            # scalar (NP, before its table load), gpsimd (rest).
            c1 = min(2, t1e)
            nc.sync.dma_start(Ws[:, : c1 * 128], wd.ap()[:, : c1 * 128])
            nc.scalar.dma_start(NP[:], npd.ap())
            nc.sync.dma_start(P[:], pd.ap())
            c2 = min(13, t1e)
            if c2 > c1:
                nc.scalar.dma_start(
                    Ws[:, c1 * 128 : c2 * 128],
                    wd.ap()[:, c1 * 128 : c2 * 128],
                )
            nc.gpsimd.dma_start(D0[:], d0d.ap())
            if t1e > c2:
                nc.gpsimd.dma_start(
                    Ws[:, c2 * 128 :], wd.ap()[:, c2 * 128 :]
                )
            nc.gpsimd.dma_start(
                Sh[:].rearrange("k (t m) -> k t m", t=4),
                shd.ap().rearrange("t k m -> k t m"),
            )

            # --- ACT table warm (Ln first: its set covers Sign/Copy) ---
            warm = wk.tile([128, 1], f32, tag="warm")
            nc.vector.memset(warm[:], 1.0)
            warm2 = wk.tile([128, 1], f32, tag="warm2")
            nc.scalar.activation(
                warm2[:], warm[:], mybir.ActivationFunctionType.Ln
            )
            lnbias = st.tile([128, 1], f32, name="lnbias")
            nc.gpsimd.memset(lnbias[:], 1e-30)

            # --- PE HAM warm-up: a few junk matmuls that finish before the
            # first real matmul's inputs land (PE queue is FIFO) ---
            junk = st.tile([128, 512], bf16, name="junk")
            nc.vector.memset(junk[:], 0.0)
            for _ in range(5):
                jps = ps_pool.tile([128, 512], f32, tag="s")
                nc.tensor.matmul(
                    jps[:], junk[:, 0:128], junk[:], start=True, stop=True
                )



            Pv = P[:].rearrange("p (c w) -> p c w", c=3)
            NPv = NP[:].rearrange("p (c w) -> p c w", c=3)

            def mm_pair(ps, off, t, w):
                """ps[:, off:off+128w] += W_t @ (P[right] - P[left])."""
                wsl = Ws[:, (t - 1) * 128 : t * 128]
                r0 = PL + t
                l0 = PL - t - 1
                nc.tensor.matmul(
                    ps[:, off : off + 128 * w],
                    wsl,
                    Pv[:, 0:w, r0 : r0 + 128],
                    start=True,
                    stop=False,
                )
                nc.tensor.matmul(
                    ps[:, off : off + 128 * w],
                    wsl,
                    NPv[:, 0:w, l0 : l0 + 128],
                    start=False,
                    stop=True,
                )

            # --- phase 1 ---
            sgn = mybir.ActivationFunctionType.Sign
            Fcur = D0
            F_by_w = {}
            act_parts = []  # (folded tile, width_units)

            for kind, ts in groups:
                w = width[ts[0]]
                u = 128 * w
                if kind == "act":
                    g = len(ts)
                    stride = 512 if u > 256 else u
                    pool = pq if g * stride > 1024 else pq4
                    ps = pool.tile(
                        [128, g * stride], f32, tag=f"q{g * stride}"
                    )
                    for j, t in enumerate(ts):
                        mm_pair(ps, j * stride, t, w)
                    b = wk.tile([128, g * u], bf16, tag=f"b{ts[0]}")
                    if stride == u:
                        nc.scalar.activation(b[:], ps[:], sgn)
                    else:
                        nc.scalar.activation(
                            b[:].rearrange("p (g w) -> p g w", g=g),
                            ps[:].rearrange("p (g w) -> p g w", g=g)[
                                :, :, 0:u
                            ],
                            sgn,
                        )
                    if u == 128:
                        # w1 groups: defer, sub-signs get PE-accumulated
                        act_parts.append((b, -g))
                        continue
                    # fold to one u-wide partial (DVE: GpS is ~2.3x slower
                    # and its serial chain gated the tail)
                    feng = nc.vector
                    if g == 1:
                        cur = b
                    elif g == 2:
                        cur = wk.tile([128, u], bf16, tag=f"f{ts[0]}_0")
                        feng.tensor_tensor(
                            cur[:], b[:, 0:u], b[:, u : 2 * u], op=alu.add
                        )
                    elif g == 3:
                        f0 = wk.tile([128, u], bf16, tag=f"f{ts[0]}_0")
                        feng.tensor_tensor(
                            f0[:], b[:, u : 2 * u], b[:, 2 * u : 3 * u],
                            op=alu.add,
                        )
                        cur = wk.tile([128, u], bf16, tag=f"f{ts[0]}_1")
                        feng.tensor_tensor(
                            cur[:], b[:, 0:u], f0[:], op=alu.add
                        )
                    else:  # g == 4
                        f0 = wk.tile([128, 2 * u], bf16, tag=f"f{ts[0]}_0")
                        feng.tensor_tensor(
                            f0[:], b[:, 0 : 2 * u], b[:, 2 * u : 4 * u],
                            op=alu.add,
                        )
                        cur = wk.tile([128, u], bf16, tag=f"f{ts[0]}_1")
                        feng.tensor_tensor(
                            cur[:], f0[:, 0:u], f0[:, u : 2 * u], op=alu.add
                        )
                    act_parts.append((cur, w))
                else:
                    t = ts[0]
                    ps = ps_pool.tile([128, 512], f32, tag="s")
                    mm_pair(ps, 0, t, w)
                    nxt = wk.tile([128, u], bf16, tag=f"Fc{t}")
                    nc.vector.scalar_tensor_tensor(
                        nxt[:],
                        ps[:, 0:u],
                        0.0,
                        Fcur[:, 0:u],
                        op0=alu.is_gt,
                        op1=alu.add,
                    )
                    Fcur = nxt
                    F_by_w[w] = nxt

            F384 = F_by_w.get(3, D0)
            F256 = F_by_w.get(2, F384)
            F128 = F_by_w.get(1, F256)
            parts3 = [p for p, pw in act_parts if pw == 3]
            parts2 = [p for p, pw in act_parts if pw == 2]
            parts1 = [p for p, pw in act_parts if pw == 1]
            w1subs = [
                (p, -pw) for p, pw in act_parts if pw < 0
            ]  # (tile of g compact 128-wide sub-signs, g)

            # --- merge: presum ACT parts per slab range, then one add with
            # the chain state writes straight into Fg slices ---
            def tree(tiles, lo, hi, ei, tag):
                eng = [nc.vector, nc.vector]
                cur = [t[:, lo:hi] for t in tiles]
                k = 0
                while len(cur) > 1:
                    nxt = []
                    for i in range(0, len(cur) - 1, 2):
                        o = wk.tile(
                            [128, hi - lo], bf16, tag=f"{tag}{k}_{i}"
                        )
                        eng[(ei + i // 2) % 2].tensor_tensor(
                            o[:], cur[i], cur[i + 1], op=alu.add
                        )
                        nxt.append(o[:])
                    if len(cur) % 2:
                        nxt.append(cur[-1])
                    cur = nxt
                    k += 1
                return cur[0]

            Fg = st.tile([128, 384], bf16, name="Fg")
            # slab2 (cols 256:384): chain F384 + 384-wide parts
            E2 = tree(parts3, 256, 384, 0, "e2") if parts3 else None
            if E2 is not None:
                nc.vector.tensor_tensor(
                    Fg[:, 256:384], F384[:, 256:384], E2, op=alu.add
                )
            else:
                nc.vector.tensor_copy(Fg[:, 256:384], F384[:, 256:384])
            # slabs 0+1 share the ACT-part sum (parts1 is empty: w1 is on
            # the chain); E01 over cols 0:256 is ready by mid-run
            E01 = (
                tree(parts3 + parts2, 0, 256, 1, "e01")
                if parts3 + parts2
                else None
            )
            if E01 is not None:
                nc.vector.tensor_tensor(
                    Fg[:, 128:256], F256[:, 128:256], E01[:, 128:256],
                    op=alu.add,
                )
            else:
                nc.vector.tensor_copy(Fg[:, 128:256], F256[:, 128:256])
            if w1subs:
                # slab0: PE-accumulate w1 sub-signs + E01 + chain into PSUM
                I0 = Sh[:, 256:384]
                psF = ps_pool.tile([128, 512], f32, tag="s")
                first = True
                for b, g in w1subs:
                    for j in range(g):
                        nc.tensor.matmul(
                            psF[:, 0:128],
                            I0,
                            b[:, j * 128 : (j + 1) * 128],
                            start=first,
                            stop=False,
                        )
                        first = False
                if E01 is not None:
                    nc.tensor.matmul(
                        psF[:, 0:128], I0, E01[:, 0:128],
                        start=first, stop=False,
                    )
                    first = False
                nc.tensor.matmul(
                    psF[:, 0:128], I0, F128[:, 0:128],
                    start=first, stop=True,
                )
                nc.vector.tensor_scalar_add(
                    Fg[:, 0:128], psF[:, 0:128], 0.0
                )
            elif E01 is not None:
                nc.vector.tensor_tensor(
                    Fg[:, 0:128], F128[:, 0:128], E01[:, 0:128], op=alu.add
                )
            else:
                nc.vector.tensor_copy(Fg[:, 0:128], F128[:, 0:128])

            # --- phase 2 (single group, width 384) ---
            ln_fn = mybir.ActivationFunctionType.Ln
            cp_fn = mybir.ActivationFunctionType.Copy
            FW2 = 3 * 130
            Fp = st.tile([128, FW2], bf16, name="Fp")
            Fpv = Fp[:].rearrange("p (c w) -> p c w", c=3)
            Fgv = Fg[:].rearrange("p (c w) -> p c w", c=3)
            nc.vector.tensor_copy(Fpv[:, :, 1:129], Fgv)
            # edge columns replicate Fg cols 0/127 (reads Fg, not Fp, so it
            # runs in parallel with the pad copy)
            nc.scalar.activation(
                Fpv[:, :, 0:130:129], Fgv[:, :, 0:128:127], cp_fn
            )
            psU = ps_pool.tile([128, 512], f32, tag="s")
            nc.tensor.matmul(
                psU[:, 0:FW2], Sh[:, 0:128], Fp[:], start=True, stop=True
            )
            psD = ps_pool.tile([128, 512], f32, tag="s")
            nc.tensor.matmul(
                psD[:, 0:FW2], Sh[:, 128:256], Fp[:], start=True, stop=True
            )
            DU = st.tile([128, FW2], bf16, name="DU")
            nc.scalar.activation(DU[:], psU[:, 0:FW2], cp_fn)
            DD = st.tile([128, FW2], bf16, name="DD")
            nc.vector.tensor_scalar_add(DD[:], psD[:, 0:FW2], 0.0)
            DUv = DU[:].rearrange("p (c w) -> p c w", c=3)
            DDv = DD[:].rearrange("p (c w) -> p c w", c=3)

            def cmp(tp, name):
                o = wk.tile([128, 384], bf16, tag=name)
                nc.vector.tensor_tensor(
                    o[:].rearrange("p (c w) -> p c w", c=3),
                    tp,
                    Fgv,
                    op=alu.is_gt,
                )
                return o

            mL = cmp(Fpv[:, :, 0:128], "mL")
            mR = cmp(Fpv[:, :, 2:130], "mR")
            mU = cmp(DUv[:, :, 1:129], "mU")
            mD = cmp(DDv[:, :, 1:129], "mD")
            mUL = cmp(DUv[:, :, 0:128], "mUL")
            mUR = cmp(DUv[:, :, 2:130], "mUR")
            mDL = cmp(DDv[:, :, 0:128], "mDL")
            mDR = cmp(DDv[:, :, 2:130], "mDR")
            # sst = r*(mL+mR+mU+mD) + (mUL+mUR+mDL+mDR), accumulated on PE
            psS = ps_pool.tile([128, 512], f32, tag="s")
            rI = Sh[:, 384:512]
            I = Sh[:, 256:384]
            for j, m in enumerate([mL, mR, mU, mD]):
                nc.tensor.matmul(
                    psS[:, 0:384], rI, m[:], start=(j == 0), stop=False
                )
            for j, m in enumerate([mUL, mUR, mDL, mDR]):
                nc.tensor.matmul(
                    psS[:, 0:384], I, m[:], start=False, stop=(j == 3)
                )
            lnS = wk.tile([128, 384], f32, tag="lnS")
            nc.scalar.activation(
                lnS[:], psS[:, 0:384], ln_fn, bias=lnbias[:], scale=float(W2)
            )
            ut = wk.tile([128, 384], f32, tag="ut")
            nc.vector.scalar_tensor_tensor(
                ut[:], lnS[:], -H_PARAM, Fg[:],
                op0=alu.mult, op1=alu.subtract,
            )
            ov = wk.tile([128, 384], f32, tag="ov")
            nc.vector.scalar_tensor_tensor(
                ov[:], psS[:, 0:384], 0.0, ut[:],
                op0=alu.is_gt, op1=alu.mult,
            )
            nc.sync.dma_start(
                outd.ap().rearrange("c h w -> h c w"),
                ov[:].rearrange("p (c w) -> p c w", c=3),
            )

    nc.compile()
    return nc


def _get_program(key):
    if key not in _PROGRAM_CACHE:
        _PROGRAM_CACHE[key] = _build(*key)
    return _PROGRAM_CACHE[key]


def _prep_core(imgs, t3e, t2e, t1e):
    """imgs: [3,128,128] binary f32, slab-ordered. Returns input dict."""
    import ml_dtypes

    PL = t1e + 1
    WIM = PL + 128 + t1e
    x = (imgs > 0).astype(np.float64)
    Pr = np.cumsum(x, axis=-1)
    Ppad = np.zeros((3, 128, WIM), dtype=np.float64)
    Ppad[:, :, PL : PL + 128] = Pr
    Ppad[:, :, PL + 128 :] = Pr[:, :, 127:128]
    P8 = np.ascontiguousarray(
        Ppad.transpose(1, 0, 2).reshape(128, 3 * WIM)
    ).astype(np.float32)
    Ts = np.array([t1e, t2e, t3e], dtype=np.float64)
    D0 = x - Ts[:, None, None]
    D0 = np.ascontiguousarray(
        D0.transpose(1, 0, 2).reshape(128, 384)
    ).astype(ml_dtypes.bfloat16)
    return {
        "p": P8.astype(ml_dtypes.float8_e4m3fn),
        "np": (-P8).astype(ml_dtypes.float8_e4m3fn),
        "d0": D0,
        "w": _make_wmats(t1e).astype(ml_dtypes.float8_e4m3fn),
        "sh": _make_shifts().astype(ml_dtypes.bfloat16),
    }


def kernel(image):
    from concourse.bass_utils import run_bass_kernel_spmd

    image = np.ascontiguousarray(np.asarray(image), dtype=np.float32)
    assert image.shape == (8, 3, 128, 128)
    flat = image.reshape(24, 128, 128)
    ns = _needed_iters_per_image(flat)
    if ns.max() == 0:
        return np.zeros_like(image)
    order = np.argsort(-ns, kind="stable")
    t3e = int(ns[order[16]]) - 1
    t2e = int(ns[order[8]]) - 1
    t1e = int(ns[order[0]]) - 1
    t3e = max(t3e, 0)
    t2e = max(t2e, t3e)
    t1e = max(t1e, max(t2e, 1))
    nc = _get_program((t3e, t2e, t1e))
    in_maps = []
    for c in range(8):
        idx = [order[c], order[8 + c], order[16 + c]]
        in_maps.append(_prep_core(flat[idx], t3e, t2e, t1e))
    res = run_bass_kernel_spmd(nc, in_maps, core_ids=list(range(8)))
    out = np.zeros((24, 128, 128), dtype=np.float32)
    for c in range(8):
        o = res.results[c]["out"].astype(np.float32)
        for s in range(3):
            out[order[8 * s + c]] = o[s]
    return out.reshape(8, 3, 128, 128)
